# revision 109
# baseline (speedup 1.0000x reference)
"""CoPE attention (nn_Attention_81922206204606) Trainium2 Bass kernel, v3.

Sharding: 16 heads over 8 cores (2 heads/core). Full inputs in, full output out.

v3 structure (330us -> 192us on the Tile cost-model timeline):
  - x/weights land in SBUF as bf16 via gpsimd casting DMAs (no staging or
    convert ops); band key-blocks (nb 14,15) load FIRST so a dedicated mini
    k-projection produces kTr ~5us in and the band pipeline starts early.
  - x^T via PE transposes batched 8-per-PSUM-bank with one batched drain;
    all k-projections emitted before q/v (main QK needs the full kT).
  - main region: QK^T pairs into 2-bank f32 PSUM slots (each matmul on a
    bank base -- mid-bank matmul writes are illegal), one Exp per pair
    drains straight into the attn^T strip; no CoPE term needed (the
    clamped-CoPE row bias cancels in softmax; the band re-adds the delta).
  - CoPE band (last W keys, reversed): QK and E-table matmuls alternate
    through one 2-slot PSUM ring (banks freed early by tanh/ssim/Et
    drains). Sigmoid via tanh (shares the Exp ACT table), floor via
    round(P-0.4999) (exact at the clamp, continuous at crossings), i16
    crossing chain, f16 scan outputs, scatters + w-interp on GPSIMD,
    deep band pool (bufs=8) for chain concurrency.
  - band attn^T enters strips via 8 PE transposes batched into one bank +
    2 batched drains per (i-chunk, head); AV/out-proj/transposes share a
    2-slot PSUM ring; Z reciprocal reads PSUM directly and is broadcast
    by gpsimd.partition_broadcast.
  - band batches interleave with main i-chunks in emission order; bf16
    partials + out, per-core out-proj + b_out/8, ReduceScatter(add) ->
    each core's 256-row slice of the output.
"""
import numpy as np

N = 2048
D = 1024
NH = 16
DH = 64
W = 160          # band width (144 crashes neuronxcc - investigating)
SW = W + 16      # band stride: tile at k*SW..+W, 16-col gap AFTER
                 # (keeps every gpsimd operand offset 32B-aligned)
NCORES = 8
SCALE = DH ** -0.5
MAIN = N - W     # 1904 columns handled by the clamp shortcut
TAIL = 15 * 128 - MAIN   # 16 band keys falling in jb14 (partitions P-TAIL..P)
NB = N // 128    # 16 key blocks
NBT = N // 128   # band tiles per head (128 rows each)
ECOLS = 130      # pos_ext columns: [dE(64), dE2(64), E0-E63, dE1]


def build_nc():
    import concourse.bass as bass
    import concourse.bacc as bacc
    import concourse.mybir as mybir
    import concourse.tile as tile

    F32 = mybir.dt.float32
    F16 = mybir.dt.float16
    I16 = mybir.dt.int16
    BF16 = mybir.dt.bfloat16
    A = mybir.AluOpType
    ACTF = mybir.ActivationFunctionType
    P = 128

    nc = bacc.Bacc(None, target_bir_lowering=False)
    x_in = nc.declare_dram_parameter("x", [N, D], F32, isOutput=False)
    wq_in = nc.declare_dram_parameter("wq", [D, P], F32, isOutput=False)
    wk_in = nc.declare_dram_parameter("wk", [D, P], F32, isOutput=False)
    wv_in = nc.declare_dram_parameter("wv", [D, P], F32, isOutput=False)
    wo_in = nc.declare_dram_parameter("wo", [P, D], F32, isOutput=False)
    bo_in = nc.declare_dram_parameter("bo", [1, D], F32, isOutput=False)
    posx_in = nc.declare_dram_parameter("posx", [P, ECOLS], F32, isOutput=False)
    iota_in = nc.declare_dram_parameter("iotaw", [P, W], F16, isOutput=False)
    iota64_in = nc.declare_dram_parameter("iota64", [P, 64], F32, isOutput=False)
    ident_in = nc.declare_dram_parameter("ident", [P, P], BF16, isOutput=False)
    out_ext = nc.declare_dram_parameter("out", [N // NCORES, D], BF16, isOutput=True)

    partial_dram = nc.dram_tensor("partial", [N, D], BF16)
    rs_dram = nc.dram_tensor("rs_out", [N // NCORES, D], BF16)

    with tile.TileContext(nc) as tc:
        import contextlib
        ctx = contextlib.ExitStack()
        with ctx:
            cpool = ctx.enter_context(tc.tile_pool(name="consts", bufs=1))
            persist = ctx.enter_context(tc.tile_pool(name="persist", bufs=1))
            work = ctx.enter_context(tc.tile_pool(name="work", bufs=3))
            band = ctx.enter_context(tc.tile_pool(name="band", bufs=3))
            etab = ctx.enter_context(tc.tile_pool(name="etab", bufs=8))
            late = ctx.enter_context(tc.tile_pool(name="late", bufs=1))
            psB = ctx.enter_context(tc.tile_pool(name="psB", bufs=2, space="PSUM"))
            xctx = contextlib.ExitStack()
            xpool = xctx.enter_context(tc.tile_pool(name="xpool", bufs=1))
            xwork = xctx.enter_context(tc.tile_pool(name="xwork", bufs=2))
            xload = xctx.enter_context(tc.tile_pool(name="xload", bufs=2))
            # phase 0-2 PSUM pools (released before the attention pools open)
            psX = xctx.enter_context(tc.tile_pool(name="psX", bufs=3, space="PSUM"))
            psP = xctx.enter_context(tc.tile_pool(name="psP", bufs=2, space="PSUM"))

            import os as _os
            _skip_cc = _os.environ.get("KERNEL_NO_CC") is not None
            _no_band = _os.environ.get("KERNEL_NO_BAND") is not None
            _no_main = _os.environ.get("KERNEL_NO_MAIN") is not None
            _no_bovr = _os.environ.get("KERNEL_NO_BANDOVR") is not None

            # ---- constants ----
            ident = cpool.tile([P, P], BF16)
            nc.sync.dma_start(ident[:], ident_in[:])
            iota_w = cpool.tile([P, W], F16)
            nc.sync.dma_start(iota_w[:], iota_in[:])
            iota64 = cpool.tile([P, 64], F16)
            nc.gpsimd.dma_start(iota64[:], iota64_in[:])
            c63 = cpool.tile([P, W], F32)
            nc.vector.memset(c63[:], 63.0)
            zW = cpool.tile([P, W], F32)
            nc.vector.memset(zW[:], 0.0)
            ones1x64 = cpool.tile([1, 64], F32)
            nc.vector.memset(ones1x64[:], 1.0)
            ones1x128 = cpool.tile([1, P], BF16)
            nc.vector.memset(ones1x128[:], 1.0)
            half128 = cpool.tile([P, 1], F32)
            nc.vector.memset(half128[:], 0.5)

            posx32 = xwork.tile([P, ECOLS], F32, tag="w32", name="posx32")
            nc.sync.dma_start(posx32[:], posx_in[:])
            posx = cpool.tile([P, ECOLS], BF16)
            nc.vector.tensor_copy(out=posx[:], in_=posx32[:])

            bo32 = cpool.tile([1, D], F32)
            nc.sync.dma_start(bo32[:], bo_in[:])
            bo_b = cpool.tile([1, D], BF16)      # b_out / 8 (summed by RS)
            nc.vector.tensor_scalar(bo_b[:], bo32[:], 1.0 / NCORES, None, A.mult)

            # weights -> bf16, D on partitions
            def load_w(src, name):
                w32 = xwork.tile([P, D // P, P], F32, tag="w32", name="w32")
                nc.sync.dma_start(w32[:], src.rearrange("(o p) f -> p o f", p=P))
                wb = xpool.tile([P, D // P, P], BF16, tag=f"wb_{name}")
                nc.vector.tensor_copy(out=wb[:], in_=w32[:])
                return wb

            wq_sb = load_w(wq_in, "q")
            wk_sb = load_w(wk_in, "k")
            wv_sb = load_w(wv_in, "v")

            wo32 = xwork.tile([P, D], F32, tag="w32", name="wo32")
            nc.sync.dma_start(wo32[:], wo_in[:])
            wo_sb = persist.tile([P, D], BF16)
            nc.vector.tensor_copy(out=wo_sb[:], in_=wo32[:])

            # ---- phases 1+2 interleaved: xT transposes + q/k/v projections ----
            # Per 512-row group g: transpose 4 x-blocks (8 transposes batched
            # into one PSUM bank, one batched drain each), then the k/q/v
            # projection matmuls for that group, then the v_nat transposes.
            DB = D // P
            xT = xpool.tile([P, DB, N], BF16)       # [d-part, d-chunk, n]
            xb_all = xpool.tile([P, NB, D], BF16)
            # gpsimd-initiated DMAs cast f32->bf16 in flight: x lands in
            # SBUF as bf16 with no staging or convert ops.
            _load_order = [14, 15] + list(range(14))
            for nb in _load_order:
                nc.gpsimd.dma_start(xb_all[:, nb, :],
                                    x_in[nb * P:(nb + 1) * P, :])

            qT = persist.tile([P, N], BF16, tag="T_q")
            kT = persist.tile([P, N], BF16, tag="T_k")
            vT = persist.tile([P, N], BF16, tag="T_v")
            vn0 = persist.tile([P, NB, 65], BF16, tag="vnat0")
            vn1 = persist.tile([P, NB, 65], BF16, tag="vnat1")
            v_nat = [vn0, vn1]

            def transpose_block(nb):
                ps_t = psX.tile([P, DB * P], BF16, tag="bT", name="ps_t")
                for dc in range(DB):
                    nc.tensor.matmul(ps_t[:, dc * P:(dc + 1) * P],
                                     xb_all[:, nb, dc * P:(dc + 1) * P],
                                     ident[:], is_transpose=True,
                                     start=(dc == 0), stop=(dc == DB - 1))
                dst = xT[:, :, nb * P:(nb + 1) * P]
                src = ps_t[:].rearrange("p (c n) -> p c n", c=DB)
                if nb % 4 == 3:
                    nc.scalar.copy(out=dst, in_=src)
                else:
                    nc.vector.tensor_copy(out=dst, in_=src)

            # band key blocks first: kTr is ready ~5us in, so the CoPE band
            # pipeline (DVE-heavy) overlaps the whole projection front.
            transpose_block(14)
            transpose_block(15)
            kband = psP.tile([P, 512], F32, tag="proj", name="kband")
            for dc in range(DB):
                nc.tensor.matmul(kband[:, 0:W], wk_sb[:, dc, :],
                                 xT[:, dc, MAIN:N],
                                 start=(dc == 0), stop=(dc == DB - 1))
            kbs = work.tile([P, W], BF16, tag="kbs")
            nc.vector.tensor_copy(out=kbs[:], in_=kband[:, 0:W])
            kTr = persist.tile([P, W], BF16)
            nc.vector.tensor_copy(out=kTr[0:DH, :], in_=kbs[0:DH, ::-1])
            nc.vector.tensor_copy(out=kTr[DH:P, :], in_=kbs[DH:P, ::-1])

            # ---- phase 3: CoPE band, all 32 tiles (2 tiles per PSUM batch) ----
            # battn_all[h][:, t, :]: exp'd band attn (natural j) for rows
            # [t*128, (t+1)*128) of head h.
            battn_h0 = late.tile([P, NBT, W], BF16, tag="battn0")
            battn_h1 = late.tile([P, NBT, W], BF16, tag="battn1")
            battn_all = [battn_h0, battn_h1]
            NT = NBT * 2
            batches = []
            pos = 0
            while pos < NT:
                batches.append(list(range(pos, min(pos + 2, NT))))
                pos += 2

            def emit_band_batch(tiles, tail_hint=False):
                B = len(tiles)
                # QK and E-table matmuls alternate through one 2-slot PSUM
                # ring; each bank is freed early (tanh+ssim / Et drains come
                # right after the matmul) so band chains overlap freely.
                ps_bs, ps_es = [], []
                for k, t in enumerate(tiles):
                    h, r = t % 2, (t // 2) * P
                    qslc = qT[h * DH:(h + 1) * DH, r:r + P]
                    ps_b = psB.tile([P, W], F32, tag="bandqk",
                                    name=f"psb{k}")
                    nc.tensor.matmul(ps_b[:], qslc,
                                     kTr[h * DH:(h + 1) * DH, :],
                                     start=True, stop=True)
                    ps_e = psB.tile([P, ECOLS], F32, tag="bandqk",
                                    name=f"pse{k}")
                    nc.tensor.matmul(ps_e[:], qslc,
                                     posx[h * DH:(h + 1) * DH, :],
                                     start=True, stop=True)
                    ps_bs.append(ps_b)
                    ps_es.append(ps_e)
                # E table cols: 0: E0-E63, 1: dE1, 2:66: dE, 66:130: dE2
                # (x8 = 1/SCALE, baked in on the host).  144-col stride keeps
                # the gpsimd slice offsets 32B-aligned.
                Et = etab.tile([P, 2, 144], F16, tag="Et")
                T_ws = band.tile([P, 2, W], F16, tag="T")
                ssim = band.tile([P, 2, W], F16, tag="ssim")
                for k in range(B):
                    nc.vector.tensor_copy(out=Et[:, k, 0:ECOLS],
                                          in_=ps_es[k][:])
                    nc.scalar.activation(T_ws[:, k, :], ps_bs[k][:],
                                         ACTF.Tanh, scale=SCALE * 0.5)
                    nc.scalar.copy(out=ssim[:, k, :], in_=ps_bs[k][:])
                # G = 0.5*T + 0.5; P = clamped prefix sum of G (per tile)
                G_ws = band.tile([P, 2, W], F16, tag="G")
                nc.vector.tensor_scalar(G_ws[:, 0:B, :], T_ws[:, 0:B, :],
                                        0.5, 0.5, A.mult, A.add)
                Pt = band.tile([P, 2, W], F32, tag="P")
                for k in range(B):
                    nc.vector.tensor_tensor_scan(
                        Pt[:, k, :], G_ws[:, k, :], c63[:], 0.0, A.add, A.min)
                # floor = round(P - 0.4999): exact at the P==63.0 clamp, and
                # a boundary miss at integer P is harmless (cope continuous).
                Fi16 = band.tile([P, 2, W], I16, tag="Fi16")
                nc.vector.tensor_scalar(Fi16[:, 0:B, :], Pt[:, 0:B, :],
                                        0.4999, None, A.subtract)
                w16 = band.tile([P, 2, W], F16, tag="w")
                nc.vector.tensor_tensor(w16[:, 0:B, :], Pt[:, 0:B, :],
                                        Fi16[:, 0:B, :], A.subtract)
                # crossings on the i16 floor: newt[j] = F[j] > F[j-1]; col 0
                # handled via the si16 memset below.
                newt = band.tile([P, 2, W], I16, tag="newt")
                nc.vector.tensor_tensor(newt[:, 0:B, 1:], Fi16[:, 0:B, 1:],
                                        Fi16[:, 0:B, :-1], A.is_gt)
                # si = (F+1)*newt - 1 as i16 scatter indices
                si_f = band.tile([P, 2, W], I16, tag="sif")
                nc.vector.scalar_tensor_tensor(si_f[:, 0:B, 1:],
                                               Fi16[:, 0:B, 1:], 1.0,
                                               newt[:, 0:B, 1:], A.add, A.mult)
                si16 = band.tile([P, 2, W], I16, tag="si16")
                nc.vector.tensor_scalar(si16[:, 0:B, 1:], si_f[:, 0:B, 1:],
                                        1.0, None, A.subtract)
                for k in range(B):
                    nc.vector.memset(si16[:, k, 0:1], -1)
                # cpos[t] = band position where F first reaches t
                cpos = band.tile([P, 2 * 64], F16, tag="cpos")
                maskF = band.tile([P, 2 * 64], F16, tag="maskF")
                for k in range(B):
                    nc.gpsimd.local_scatter(cpos[:, k * 64:(k + 1) * 64],
                                            iota_w[:], si16[:, k, :],
                                            channels=P, num_elems=64,
                                            num_idxs=W)
                    # t <= floor(P) iff t <= P for integer t: use Pt directly
                    nc.vector.tensor_scalar(maskF[:, k * 64:(k + 1) * 64],
                                            iota64[:], Pt[:, k, W - 1:W],
                                            None, A.is_le)
                cpm = band.tile([P, 2 * 64], F16, tag="cpm")
                nc.vector.scalar_tensor_tensor(cpm[:, 0:B * 64],
                                               cpos[:, 0:B * 64], 1.0,
                                               maskF[:, 0:B * 64],
                                               A.add, A.mult)
                cpm16 = band.tile([P, 2 * 64], I16, tag="cpm16")
                nc.vector.tensor_scalar(cpm16[:, 0:B * 64], cpm[:, 0:B * 64],
                                        1.0, None, A.subtract)
                for k in range(B):
                    nc.vector.memset(cpm16[:, k * 64:k * 64 + 1], -1)
                # scatter dE/dE2 to crossing positions, then prefix-sum
                # (the dE/dE2 scans run on GPSIMD to unload the DVE)
                dFl = band.tile([P, 2, W], F16, tag="dFl")
                dSl = band.tile([P, 2, W], F16, tag="dSl")
                Efl = band.tile([P, 2, W], F16, tag="Efl")
                Sl = band.tile([P, 2, W], F16, tag="Sl")
                for k in range(B):
                    nc.gpsimd.local_scatter(dFl[:, k, :], Et[:, k, 0:64],
                                            cpm16[:, k * 64:(k + 1) * 64],
                                            channels=P, num_elems=W,
                                            num_idxs=64)
                    nc.gpsimd.local_scatter(dSl[:, k, :], Et[:, k, 64:128],
                                            cpm16[:, k * 64:(k + 1) * 64],
                                            channels=P, num_elems=W,
                                            num_idxs=64)
                    nc.vector.tensor_tensor_scan(
                        Efl[:, k, :], dFl[:, k, :],
                        zW[:], Et[:, k, 128:129], A.add, A.add)
                    nc.vector.tensor_tensor_scan(
                        Sl[:, k, :], dSl[:, k, :],
                        zW[:], Et[:, k, 129:130], A.add, A.add)
                # logits8 = sim + Efl + w*Sl (8x scale; exp applies SCALE)
                t1 = band.tile([P, 2, W], F16, tag="t1")
                nc.vector.tensor_tensor(t1[:, 0:B, :], w16[:, 0:B, :],
                                        Sl[:, 0:B, :], A.mult)
                t2 = band.tile([P, 2, W], F16, tag="t2")
                nc.vector.tensor_tensor(t2[:, 0:B, :], t1[:, 0:B, :],
                                        Efl[:, 0:B, :], A.add)
                logits = band.tile([P, 2, W], F16, tag="lg")
                nc.vector.tensor_tensor(logits[:, 0:B, :], ssim[:, 0:B, :],
                                        t2[:, 0:B, :], A.add)
                # battn stored in NATURAL key order (chain ran reversed):
                # battn[:, ti, c] is key j = MAIN + c.  Exp reverses into a 2D
                # staging tile (v1-proven AP form), then a straight copy.
                battn2 = band.tile([P, B * W], BF16, tag="battn2")
                for k, t in enumerate(tiles):
                    h, ti = t % 2, t // 2
                    nc.scalar.activation(battn2[:, k * W:(k + 1) * W][:, ::-1],
                                         logits[:, k, :], ACTF.Exp,
                                         scale=SCALE)
                    nc.gpsimd.tensor_copy(out=battn_all[h][:, ti, :],
                                           in_=battn2[:, k * W:(k + 1) * W])


            def project(name, wb, t_out, g):
                ps = psP.tile([P, 512], F32, tag="proj", name=f"pj_{name}{g}")
                for dc in range(DB):
                    nc.tensor.matmul(ps[:], wb[:, dc, :],
                                     xT[:, dc, g * 512:(g + 1) * 512],
                                     start=(dc == 0), stop=(dc == DB - 1))
                if name == "q":
                    nc.scalar.copy(out=t_out[:, g * 512:(g + 1) * 512],
                                   in_=ps[:])
                else:
                    nc.vector.tensor_copy(
                        out=t_out[:, g * 512:(g + 1) * 512], in_=ps[:])

            # k first (the main QK needs ALL of kT), with q0/q1 slipped in
            # early so the band pipeline (needs qT rows) can start sooner
            for g in range(4):
                for nb in range(4 * g, 4 * g + 4):
                    if nb in (14, 15):
                        continue
                    transpose_block(nb)
                project("k", wk_sb, kT, g)
                if g >= 2:
                    project("q", wq_sb, qT, g - 2)
            for g in range(2, 4):
                project("q", wq_sb, qT, g)
            for g in range(4):
                project("v", wv_sb, vT, g)
                # v natural for this group's 4 jb blocks (+ ones column)
                ps_v = psX.tile([P, DB * P], BF16, tag="bT")
                for j in range(4):
                    jb = 4 * g + j
                    nc.tensor.matmul(ps_v[:, j * P:(j + 1) * P],
                                     vT[:, jb * P:(jb + 1) * P],
                                     ident[:], is_transpose=True,
                                     start=(j == 0), stop=(j == 3))
                ps_v3 = ps_v[:, 0:512].rearrange("p (j c) -> p j c", j=4)
                nc.vector.tensor_copy(out=vn0[:, 4 * g:4 * g + 4, 0:64],
                                      in_=ps_v3[:, :, 0:64])
                nc.scalar.copy(out=vn1[:, 4 * g:4 * g + 4, 0:64],
                               in_=ps_v3[:, :, 64:P])
                if not _no_band:
                    for bi in range(4 * g, 4 * g + 4):
                        emit_band_batch(batches[bi])

            nc.vector.memset(vn0[:, :, 64:65], 1.0)
            nc.vector.memset(vn1[:, :, 64:65], 1.0)

            # xT / weight staging no longer needed: release their SBUF + PSUM
            xctx.close()
            strips = ctx.enter_context(tc.tile_pool(name="strips", bufs=1))
            psMain = ctx.enter_context(tc.tile_pool(name="psMain", bufs=2, space="PSUM"))
            psD = ctx.enter_context(tc.tile_pool(name="psD", bufs=2, space="PSUM"))


            # ---- phase 4: attn^T strips + AV, 4 i-chunks of 512, with the
            # i-chunk's band batches emitted just before it (band DVE/Pool
            # work overlaps the previous chunk's PE/ACT-heavy main work) ----
            avT = late.tile([P, N], BF16)          # normalized (attn@V).T

            def emit_main_ig(ig):
                tail = ig == 3
                strip0 = strips.tile([P, NB, 512], BF16, tag="strip0")
                strip1 = strips.tile([P, NB, 512], BF16, tag="strip1")
                strip = [strip0, strip1]
                for h in range(2):
                    # main region: jb 0..14 (jb15 is all band).
                    # QK^T pairs -> two f32 PSUM banks (each matmul on a bank
                    # base) -> one exp per pair.
                    for jb0 in range(0, 15, 2):
                        npair = min(2, 15 - jb0)
                        ps = psMain.tile([P, 2, 512], F32, tag="qk")
                        for k in range(npair):
                            nc.tensor.matmul(
                                ps[:, k, :],
                                kT[h * DH:(h + 1) * DH,
                                   (jb0 + k) * P:(jb0 + k + 1) * P],
                                qT[h * DH:(h + 1) * DH,
                                   ig * 512:(ig + 1) * 512],
                                start=True, stop=True)
                        nc.scalar.activation(
                            strip[h][:, jb0:jb0 + npair, :],
                            ps[:, 0:npair, :], ACTF.Exp, scale=SCALE)
                    # band overwrite: rows r0..r0+127 for the 4 band tiles of
                    # this i-chunk; battn col c is key j = MAIN + c.
                    if _no_bovr:
                        # zero the band region: softmax restricted to j<MAIN
                        nc.vector.memset(strip[h][:, 15, :], 0.0)
                        nc.vector.memset(strip[h][P - TAIL:P, 14, :], 0.0)
                    if not (_no_band or _no_bovr):
                        # all 4 band tiles' transposes batched into ONE PSUM
                        # bank (8 transposes, each a complete group writing a
                        # disjoint region), then 2 batched drains.
                        pt4 = psD.tile([P, 4, 2 * P], BF16, tag="psav",
                                       name="pt4")
                        for bt in range(4):
                            ti = ig * 4 + bt
                            # j MAIN..1919 (battn cols 0..TAIL)
                            nc.tensor.matmul(
                                pt4[0:TAIL, bt, P:2 * P],
                                battn_all[h][:, ti, 0:TAIL],
                                ident[:], is_transpose=True, start=True,
                                stop=True)
                            # j 1920..2047 (battn cols TAIL..W)
                            nc.tensor.matmul(
                                pt4[:, bt, 0:P],
                                battn_all[h][:, ti, TAIL:W], ident[:],
                                is_transpose=True, start=True, stop=True)
                        nc.vector.tensor_copy(
                            out=strip[h][:, 15, :],
                            in_=pt4[:, :, 0:P])
                        nc.vector.tensor_copy(
                            out=strip[h][P - TAIL:P, 14, :],
                            in_=pt4[0:TAIL, :, P:2 * P])
                # AV per head: accumulate over jb; row 64 = Z
                for h in range(2):
                    ps_av = psD.tile([65, 512], F32, tag="psav")
                    for jb in range(NB):
                        nc.tensor.matmul(ps_av[:], v_nat[h][:, jb, :],
                                         strip[h][:, jb, :],
                                         start=(jb == 0), stop=(jb == NB - 1))
                    rz1 = work.tile([1, 512], F32, tag="rz1", bufs=2)
                    nc.vector.reciprocal(rz1[:], ps_av[64:65, :])
                    rzbc = work.tile([64, 512], F32, tag="rzbc", bufs=2)
                    nc.gpsimd.partition_broadcast(rzbc[:], rz1[:])
                    nc.vector.tensor_tensor(
                        avT[h * DH:(h + 1) * DH, ig * 512:(ig + 1) * 512],
                        ps_av[0:64, :], rzbc[:], A.mult)
                # partial out-proj for the 4 row-blocks this i-chunk completed
                for rb in range(ig * 4, ig * 4 + 4):
                    po = work.tile([P, D], BF16, tag="po", bufs=2)
                    for dg in range(2):
                        ps_p = psD.tile([P, 512], F32, tag="psav")
                        nc.tensor.matmul(ps_p[:], avT[:, rb * P:(rb + 1) * P],
                                         wo_sb[:, dg * 512:(dg + 1) * 512],
                                         start=True, stop=False)
                        nc.tensor.matmul(ps_p[:], ones1x128[:],
                                         bo_b[:, dg * 512:(dg + 1) * 512],
                                         start=False, stop=True)
                        nc.vector.tensor_copy(
                            out=po[:, dg * 512:(dg + 1) * 512],
                            in_=ps_p[:])
                    nc.sync.dma_start(partial_dram[rb * P:(rb + 1) * P, :],
                                      po[:])
                    if _skip_cc and rb < 2:
                        nc.scalar.dma_start(out_ext[rb * P:(rb + 1) * P, :],
                                            po[:])

            if not _no_main:
                for ig in range(4):
                    emit_main_ig(ig)

            # ---- phase 6: ReduceScatter + write out ----
            if not _skip_cc:
                nc.gpsimd.collective_compute(
                    "ReduceScatter", mybir.AluOpType.add,
                    replica_groups=[list(range(NCORES))],
                    ins=[partial_dram[:]], outs=[rs_dram[:]])
                for b in range(2):
                    t = work.tile([P, D], BF16, tag="outcp")
                    nc.sync.dma_start(t[:], rs_dram[b * P:(b + 1) * P, :])
                    nc.sync.dma_start(out_ext[b * P:(b + 1) * P, :], t[:])

    nc.compile()
    return nc


def make_posx(pos_emb):
    """pos_ext [128, 130] f32: stacked twice on partitions.
    cols: 0: E0-E63 basis, 1: dE1, 2:66: dE table (dE_0=0, dE_t=p_t-p_{t-1}),
    66:130: dE2 table (dE2_t = dE_{t+1}-dE_t, dE_64:=0)."""
    C, T = pos_emb.shape  # (64, 64)
    px = np.zeros((C, ECOLS), np.float32)
    dE = np.zeros((C, 65), np.float32)
    dE[:, 1:64] = pos_emb[:, 1:] - pos_emb[:, :-1]
    dE2 = dE[:, 1:65] - dE[:, 0:64]
    px[:, 0:64] = dE[:, 0:64]
    px[:, 64:128] = dE2
    px[:, 128] = pos_emb[:, 0] - pos_emb[:, 63]
    px[:, 129] = dE[:, 1]
    # x8 = 1/SCALE: the CoPE term is accumulated into the raw-sim PSUM bank
    # and the final exp applies scale=SCALE to the sum.
    return np.concatenate([px, px], axis=0) * 8.0


_NC_CACHE = None


def _get_nc():
    global _NC_CACHE
    if _NC_CACHE is None:
        _NC_CACHE = build_nc()
    return _NC_CACHE


def make_in_maps(inputs):
    x = np.ascontiguousarray(np.asarray(inputs["x"], dtype=np.float32).reshape(N, D))
    Wq = np.asarray(inputs["Wq"], dtype=np.float32)
    Wkv = np.asarray(inputs["Wkv"], dtype=np.float32)
    Wout = np.asarray(inputs["Wout"], dtype=np.float32)
    b_out = np.asarray(inputs["b_out"], dtype=np.float32).reshape(1, D)
    pos_emb = np.asarray(inputs["pos_emb"], dtype=np.float32)
    posx = make_posx(pos_emb)
    iotaw = np.tile(np.arange(W, dtype=np.float16), (128, 1))
    iota64 = np.tile(np.arange(64, dtype=np.float32), (128, 1))
    import ml_dtypes
    ident_bf = np.eye(128, dtype=np.float32).astype(ml_dtypes.bfloat16)
    in_maps = []
    for c in range(NCORES):
        sl = slice(128 * c, 128 * (c + 1))
        in_maps.append({
            "x": x,
            "wq": np.ascontiguousarray(Wq[:, sl]),
            "wk": np.ascontiguousarray(Wkv[:, :D][:, sl]),
            "wv": np.ascontiguousarray(Wkv[:, D:][:, sl]),
            "wo": np.ascontiguousarray(Wout[sl, :]),
            "bo": b_out,
            "posx": posx,
            "iotaw": iotaw,
            "iota64": iota64,
            "ident": ident_bf,
        })
    return in_maps


def kernel(**inputs):
    from concourse import bass_utils
    nc = _get_nc()
    in_maps = make_in_maps(inputs)
    res = bass_utils.run_bass_kernel_spmd(nc, in_maps, list(range(NCORES)))
    outs = [np.asarray(res.results[c]["out"]) for c in range(NCORES)]
    full = np.concatenate(outs, axis=0).astype(np.float32)
    return full.reshape(1, N, D)

